# revision 44
# baseline (speedup 1.0000x reference)
# Bass/Trainium2 kernel for nn_ActorCritic (scatter_memory):
#   x = zeros_like(features).at[idx].set(features[idx])  == features * mask,
#       mask[r] = (r in idx)
#   act_out  = tanh(selu(LN(cat(err,dedt,w,x) @ W1a + b1a)) @ W2a + b2a)
#   crit_out =      selu(LN(cat(err,dedt,w,act,x) @ W1c + b1c)) @ W2c + b2c
#
# 8-core data-parallel: rows sharded, weights replicated.  The membership
# mask is built on-device with a race-free TensorEngine histogram: per
# 128-token chunk of the core's idx shard, one-hot(idx&127) is the
# stationary operand and one-hot(idx>>7) the moving operand, accumulated in
# PSUM over the whole shard; a ReduceScatter(add) sums the 8 partial count
# tables and hands each core its own row range, and mask = min(count, 1).
#
# The MLP: layer-1 matmuls in float32r (full PE rate at 512-wide moving
# operand), LN bias b1 folded in via an appended ones-column, SELU via
# exp/relu/min with the lambda folded into W2, layer 2 in fp16 with the
# activations transposed by the DMA xbar.  LN sqrt is batched per tile
# group so the ACT engine loads the sqrt table twice per group instead of
# per tile.
import numpy as np

import concourse.mybir as mybir
import concourse.tile as tile
from concourse import bacc
from concourse.bass_utils import run_bass_kernel_spmd
from concourse.masks import make_identity

F32 = mybir.dt.float32
F32R = mybir.dt.float32r
F16 = mybir.dt.float16
F8 = mybir.dt.float8e4
BF16 = mybir.dt.bfloat16
I32 = mybir.dt.int32
I16 = mybir.dt.int16

SELU_ALPHA = 1.6732632423543772848170429916717
SELU_LAMBDA = 1.0507009873554804934193349852946

FEAT = 512
LIN = 512
NSMALL = 22  # err(6) dedt(6) w(3) ones(1) actions(6)
K_ACT = 16   # actor uses rows 0:16 of smallsT (incl. ones)
K_CRIT = 22  # critic uses all 22

_BUILD_CACHE = {}


def build_nc(n_total, n_cores, group_tiles=8, general_ln=False):
    """Build the SPMD Bacc graph for one core (identical across cores)."""
    R = n_total // n_cores          # rows per core
    assert R % 128 == 0
    T = R // 128                    # 128-row tiles per core
    B = R // 128                    # idx-shard chunks (tokens laid [128, B])
    G = min(group_tiles, T)
    assert T % G == 0
    NG = T // G
    GC = G * 128                    # columns per group

    nc = bacc.Bacc()

    # ---------------- DRAM parameters ----------------
    featT = nc.dram_tensor("featT", [FEAT, R], F32, kind="ExternalInput")
    smallsT = nc.dram_tensor("smallsT", [NSMALL, R], F32, kind="ExternalInput")
    idx_tok = nc.dram_tensor("idx_tok", [128, B], I32, kind="ExternalInput")
    w1a_big = nc.dram_tensor("w1a_big", [FEAT, LIN], F32, kind="ExternalInput")
    w1c_big = nc.dram_tensor("w1c_big", [FEAT, LIN], F32, kind="ExternalInput")
    w1a_small = nc.dram_tensor("w1a_small", [K_ACT, LIN], F32, kind="ExternalInput")
    w1c_small = nc.dram_tensor("w1c_small", [K_CRIT, LIN], F32, kind="ExternalInput")
    w2a_in = nc.dram_tensor("w2a", [LIN, 6], F32, kind="ExternalInput")
    w2c_in = nc.dram_tensor("w2c", [LIN, 1], F32, kind="ExternalInput")
    b2a_in = nc.dram_tensor("b2a", [6], F32, kind="ExternalInput")
    b2c_in = nc.dram_tensor("b2c", [1], F32, kind="ExternalInput")
    g1a_in = nc.dram_tensor("g1a", [LIN], F32, kind="ExternalInput")
    be1a_in = nc.dram_tensor("be1a", [LIN], F32, kind="ExternalInput")
    g1c_in = nc.dram_tensor("g1c", [LIN], F32, kind="ExternalInput")
    be1c_in = nc.dram_tensor("be1c", [LIN], F32, kind="ExternalInput")

    out7 = nc.dram_tensor("out7", [7, R], F32, kind="ExternalOutput")

    # histogram table (Local) and reduce-scatter output (this core's rows)
    cnt_tbl = nc.dram_tensor("cnt_tbl", [n_total], F32)
    cnt_rs = nc.dram_tensor("cnt_rs", [R], F32)

    with tile.TileContext(nc) as tc:
        with (
            tc.tile_pool(name="consts", bufs=1) as consts,
        ):
            # ---- weights / constants ----
            wa = consts.tile([128, 4, LIN], F32R, tag="wa")
            nc.gpsimd.dma_start(out=wa[:], in_=w1a_big.ap().rearrange("(f p) n -> p f n", p=128))
            wc = consts.tile([128, 4, LIN], F32R, tag="wc")
            nc.gpsimd.dma_start(out=wc[:], in_=w1c_big.ap().rearrange("(f p) n -> p f n", p=128))
            wsa = consts.tile([K_ACT, LIN], F32R, tag="wsa")
            nc.gpsimd.dma_start(out=wsa[:], in_=w1a_small[:, :])
            wsc = consts.tile([K_CRIT, LIN], F32R, tag="wsc")
            nc.gpsimd.dma_start(out=wsc[:], in_=w1c_small[:, :])
            w2a = consts.tile([128, 4, 6], F16, tag="w2a")
            nc.gpsimd.dma_start(out=w2a[:], in_=w2a_in.ap().rearrange("(f p) j -> p f j", p=128))
            nc.vector.tensor_scalar_mul(w2a[:], w2a[:], SELU_LAMBDA)
            w2c = consts.tile([128, 4, 1], F16, tag="w2c")
            nc.gpsimd.dma_start(out=w2c[:], in_=w2c_in.ap().rearrange("(f p) j -> p f j", p=128))
            nc.vector.tensor_scalar_mul(w2c[:], w2c[:], SELU_LAMBDA)
            b2a = consts.tile([6, 1], F32, tag="b2a")
            nc.gpsimd.dma_start(out=b2a[:], in_=b2a_in[:, None])
            b2c = consts.tile([1, 1], F32, tag="b2c")
            nc.gpsimd.dma_start(out=b2c[:], in_=b2c_in[:, None])
            epsb = consts.tile([128, 1], F32, tag="epsb")
            nc.vector.memset(epsb[:], 1e-5)

            if general_ln:
                g_a = consts.tile([128, LIN], F16, tag="g_a")
                nc.gpsimd.dma_start(out=g_a[:], in_=g1a_in[None, :].to_broadcast([128, LIN]))
                be_a = consts.tile([128, LIN], F16, tag="be_a")
                nc.gpsimd.dma_start(out=be_a[:], in_=be1a_in[None, :].to_broadcast([128, LIN]))
                g_c = consts.tile([128, LIN], F16, tag="g_c")
                nc.gpsimd.dma_start(out=g_c[:], in_=g1c_in[None, :].to_broadcast([128, LIN]))
                be_c = consts.tile([128, LIN], F16, tag="be_c")
                nc.gpsimd.dma_start(out=be_c[:], in_=be1c_in[None, :].to_broadcast([128, LIN]))

            # ---------------- phase A: membership mask ----------------
            # Race-free histogram of the idx shard over all N rows:
            # counts[lo, hi] accumulated on the TensorEngine in PSUM.
            HI = n_total // 128
            with (
                tc.tile_pool(name="phA", bufs=1) as pa,
                tc.tile_pool(name="phAw", bufs=3) as paw,
                tc.tile_pool(name="psCnt", bufs=1, space="PSUM") as psCnt,
            ):
                v = pa.tile([128, B], I32, tag="v")
                nc.gpsimd.dma_start(out=v[:], in_=idx_tok[:, :])
                vhi_i = pa.tile([128, B], I32, tag="vhi_i")
                nc.vector.tensor_scalar(
                    vhi_i[:], v[:], 7, None, mybir.AluOpType.logical_shift_right)
                vhi = pa.tile([128, B], F32, tag="vhi")
                nc.vector.tensor_copy(vhi[:], vhi_i[:])
                vlo_i = pa.tile([128, B], I32, tag="vlo_i")
                nc.vector.tensor_scalar(
                    vlo_i[:], v[:], 127, None, mybir.AluOpType.bitwise_and)
                vlo = pa.tile([128, B], F32, tag="vlo")
                nc.vector.tensor_copy(vlo[:], vlo_i[:])
                iota_hi_i = pa.tile([128, HI], I16, tag="iota_hi_i")
                nc.gpsimd.iota(iota_hi_i[:], pattern=[[1, HI]], base=0,
                               channel_multiplier=0)
                iota_hi = pa.tile([128, HI], F16, tag="iota_hi")
                nc.vector.tensor_copy(iota_hi[:], iota_hi_i[:])
                iota_lo_i = pa.tile([128, 128], I16, tag="iota_lo_i")
                nc.gpsimd.iota(iota_lo_i[:], pattern=[[1, 128]], base=0,
                               channel_multiplier=0)
                iota_lo = pa.tile([128, 128], F16, tag="iota_lo")
                nc.vector.tensor_copy(iota_lo[:], iota_lo_i[:])

                cnt_ps = psCnt.tile([128, HI], F32, tag="cnt_ps")
                for j in range(B):
                    a_t = paw.tile([128, HI], F16, tag="a_t")
                    nc.vector.tensor_scalar(
                        a_t[:], iota_hi[:], vhi[:, j:j + 1], None,
                        mybir.AluOpType.is_equal)
                    b_t = paw.tile([128, 128], F16, tag="b_t")
                    nc.vector.tensor_scalar(
                        b_t[:], iota_lo[:], vlo[:, j:j + 1], None,
                        mybir.AluOpType.is_equal)
                    MMW = min(512, HI)
                    for h in range(HI // MMW):
                        nc.tensor.matmul(
                            cnt_ps[:, h * MMW:(h + 1) * MMW],
                            lhsT=b_t[:], rhs=a_t[:, h * MMW:(h + 1) * MMW],
                            start=(j == 0), stop=(j == B - 1),
                        )
                # counts to DRAM in row-major order: transpose each 128-wide
                # hi-block so DMA runs are 512B-contiguous
                cnt_sb = pa.tile([128, HI], F32, tag="cnt_sb")
                nc.scalar.copy(cnt_sb[:], cnt_ps[:])
                identf = pa.tile([128, 128], F32, tag="identf")
                make_identity(nc, identf[:])
                HW_ = min(128, HI)
                n_h = HI // HW_
                cntT = pa.tile([HW_, n_h, 128], F32, tag="cntT")
                for h in range(n_h):
                    ctp = psCnt.tile([HW_, 128], F32, tag="ctp")
                    nc.tensor.transpose(
                        ctp[:], cnt_sb[:, h * HW_:(h + 1) * HW_], identf[:])
                    nc.vector.tensor_copy(cntT[:, h, :], ctp[:])
                nc.gpsimd.dma_start(
                    out=cnt_tbl.ap().rearrange("(h hi lo) -> hi h lo", hi=HW_, lo=128),
                    in_=cntT[:],
                )
                # sum partial counts across cores; each core keeps its row range
                nc.gpsimd.collective_compute(
                    "ReduceScatter",
                    mybir.AluOpType.add,
                    replica_groups=[list(range(n_cores))],
                    ins=[cnt_tbl.ap()],
                    outs=[cnt_rs.ap()],
                )

            # ---------------- main loop ----------------
            with (
                tc.tile_pool(name="xg", bufs=3) as xgp,
                tc.tile_pool(name="sg", bufs=2) as sgp,
                tc.tile_pool(name="og", bufs=2) as ogp,
                tc.tile_pool(name="hsb", bufs=2 * G + 2) as hsbp,
                tc.tile_pool(name="wk", bufs=2) as wk,
                tc.tile_pool(name="wks", bufs=G + 2) as wks,
                tc.tile_pool(name="psH", bufs=2, space="PSUM") as psH,
                tc.tile_pool(name="psO", bufs=2, space="PSUM") as psO,
            ):
                for g in range(NG):
                    gsl = slice(g * GC, (g + 1) * GC)
                    xg = xgp.tile([128, 4, GC], F32R, tag="xg")
                    nc.gpsimd.dma_start(
                        out=xg[:],
                        in_=featT.ap().rearrange("(f p) r -> p f r", p=128)[:, :, gsl],
                    )
                    sg = sgp.tile([NSMALL, GC], F32R, tag="sg")
                    nc.gpsimd.dma_start(out=sg[:], in_=smallsT[:, gsl])
                    oga = ogp.tile([6, GC], F32, tag="oga")
                    ogc = ogp.tile([1, GC], F32, tag="ogc")
                    # mask the x part (rows with no idx hit contribute zero):
                    # broadcast-load this group's counts, mask = min(count, 1)
                    cr = sgp.tile([128, GC], F32, tag="cr")
                    nc.sync.dma_start(
                        out=cr[:], in_=cnt_rs[None, gsl].to_broadcast([128, GC]))
                    mg = sgp.tile([128, GC], F32R, tag="mg")
                    nc.vector.tensor_scalar(
                        mg[:], cr[:], 1.0, None, mybir.AluOpType.min)
                    for f in range(4):
                        nc.vector.tensor_tensor(
                            out=xg[:, f, :], in0=xg[:, f, :], in1=mg[:],
                            op=mybir.AluOpType.mult,
                        )

                    # -- sweep 1: layer 1 + psum eviction + LN stats --
                    agg_g = wks.tile([128, 4 * G], F32, tag="agg_g")
                    hs = {}
                    for t in range(G):
                        tsl = slice(t * 128, (t + 1) * 128)
                        psA = psH.tile([128, LIN], F32, tag="psA")
                        psC = psH.tile([128, LIN], F32, tag="psC")
                        for f in range(4):
                            nc.tensor.matmul(
                                psA[:], lhsT=xg[:, f, tsl], rhs=wa[:, f, :],
                                start=(f == 0), stop=False,
                            )
                        nc.tensor.matmul(
                            psA[:], lhsT=sg[:K_ACT, tsl], rhs=wsa[:],
                            start=False, stop=True,
                        )
                        for f in range(4):
                            nc.tensor.matmul(
                                psC[:], lhsT=xg[:, f, tsl], rhs=wc[:, f, :],
                                start=(f == 0), stop=False,
                            )
                        nc.tensor.matmul(
                            psC[:], lhsT=sg[:K_CRIT, tsl], rhs=wsc[:],
                            start=False, stop=True,
                        )
                        for net, ps in (("a", psA), ("c", psC)):
                            h = hsbp.tile([128, LIN], F16, tag=f"h{net}")
                            nc.scalar.copy(h[:], ps[:])
                            st6 = wk.tile([128, 6], F32, tag=f"st6{net}")
                            nc.vector.bn_stats(st6[:], h[:])
                            off = 4 * t + (0 if net == "a" else 2)
                            nc.vector.bn_aggr(agg_g[:, off:off + 2], st6[:])
                            hs[(t, net)] = h

                    # -- sweep 2: group-wide LN scalars --
                    # sd over the strided var columns (one Sqrt instruction,
                    # so the ACT table switches only twice per group)
                    sd_g = wks.tile([128, 2 * G], F32, tag="sd_g")
                    nc.scalar.activation(
                        sd_g[:], agg_g[:].rearrange("p (t two) -> p (t two)", two=2)[
                            :, 1::2],
                        mybir.ActivationFunctionType.Sqrt,
                        bias=epsb[:], scale=1.0,
                    )
                    k_g = wks.tile([128, 2 * G], F32, tag="k_g")
                    nc.vector.reciprocal(k_g[:], sd_g[:])
                    # nmk = -mean * k for all tiles/nets at once
                    nmk_g = wks.tile([128, 2 * G], F32, tag="nmk_g")
                    nc.vector.tensor_tensor(
                        nmk_g[:], agg_g[:].rearrange("p (t two) -> p (t two)", two=2)[
                            :, 0::2],
                        k_g[:], op=mybir.AluOpType.mult)
                    nc.vector.tensor_scalar(
                        nmk_g[:], nmk_g[:], -1.0, None, mybir.AluOpType.mult)

                    # -- sweep 3: selu(LN(h)) and layer 2, two tiles per
                    # layer-2 matmul batch (N=256 moving operand) --
                    for tp in range(G // 2):
                        psl = slice(tp * 256, (tp + 1) * 256)
                        out2a = psO.tile([6, 256], F32, tag="out2a")
                        out2c = psO.tile([1, 256], F32, tag="out2c")
                        haTpa = wk.tile([128, 4, 256], F16, tag="haTpa")
                        haTpc = wk.tile([128, 4, 256], F16, tag="haTpc")
                        haTp = {"a": haTpa, "c": haTpc}
                        for ti in range(2):
                          t = 2 * tp + ti
                          for net in ("a", "c"):
                            h = hs[(t, net)]
                            col = 2 * t + (0 if net == "a" else 1)
                            k = k_g[:, col:col + 1]
                            nmk = nmk_g[:, col:col + 1]
                            ti_sl = slice(ti * 128, (ti + 1) * 128)
                            if general_ln:
                                z_t = wk.tile([128, LIN], F16, tag=f"z{net}")
                                nc.scalar.activation(
                                    z_t[:], h[:], mybir.ActivationFunctionType.Identity,
                                    bias=nmk, scale=k,
                                )
                                gg = g_a if net == "a" else g_c
                                bb = be_a if net == "a" else be_c
                                nc.vector.tensor_tensor(
                                    z_t[:], z_t[:], gg[:], op=mybir.AluOpType.mult)
                                nc.vector.tensor_tensor(
                                    z_t[:], z_t[:], bb[:], op=mybir.AluOpType.add)
                                e_t = wk.tile([128, LIN], F16, tag=f"e{net}")
                                nc.scalar.activation(
                                    e_t[:], z_t[:], mybir.ActivationFunctionType.Exp,
                                    bias=0.0, scale=1.0)
                                r_t = wk.tile([128, LIN], F16, tag=f"r{net}")
                                nc.vector.tensor_scalar(
                                    r_t[:], z_t[:], 0.0, -SELU_ALPHA,
                                    mybir.AluOpType.max, mybir.AluOpType.add)
                            else:
                                e_t = wk.tile([128, LIN], F16, tag=f"e{net}")
                                nc.scalar.activation(
                                    e_t[:], h[:], mybir.ActivationFunctionType.Exp,
                                    bias=nmk, scale=k,
                                )
                                # r = max(k*h + nmk, 0) - alpha on the DVE
                                r_t = wk.tile([128, LIN], F16, tag=f"r{net}")
                                nc.vector.tensor_scalar(
                                    r_t[:], h[:], k, nmk,
                                    mybir.AluOpType.mult, mybir.AluOpType.add)
                                nc.vector.tensor_scalar(
                                    r_t[:], r_t[:], 0.0, -SELU_ALPHA,
                                    mybir.AluOpType.max, mybir.AluOpType.add)
                            # ha/lambda = (relu - alpha) + alpha*min(e, 1)
                            nc.vector.tensor_scalar(
                                e_t[:], e_t[:], 1.0, SELU_ALPHA,
                                mybir.AluOpType.min, mybir.AluOpType.mult)
                            ha = wk.tile([128, LIN], F16, tag=f"ha{net}")
                            nc.vector.tensor_tensor(
                                ha[:], r_t[:], e_t[:], op=mybir.AluOpType.add)
                            # transpose via the DMA xbar into this tile's half
                            # of the pair buffer: haT[p, c, r] = ha[r, 128c+p]
                            nc.sync.dma_start_transpose(
                                out=haTp[net][:, :, ti_sl], in_=ha[:])
                        for net in ("a", "c"):
                            w2 = w2a if net == "a" else w2c
                            o2 = out2a if net == "a" else out2c
                            for f in range(4):
                                nc.tensor.matmul(
                                    o2[:], lhsT=w2[:, f, :], rhs=haTp[net][:, f, :],
                                    start=(f == 0), stop=(f == 3),
                                )
                        nc.scalar.activation(
                            oga[:, psl], out2a[:],
                            mybir.ActivationFunctionType.Tanh, bias=b2a[:], scale=1.0,
                        )
                        nc.scalar.activation(
                            ogc[:, psl], out2c[:],
                            mybir.ActivationFunctionType.Identity, bias=b2c[:], scale=1.0,
                        )
                    nc.sync.dma_start(out=out7[0:6, gsl], in_=oga[:])
                    nc.sync.dma_start(out=out7[6:7, gsl], in_=ogc[:])

    nc.compile()
    return nc


def _prep_host(inputs, n_cores):
    """Layout-only host prep: shard + transpose + permute, no arithmetic on
    data values (the ones column and dtype casts are the only additions)."""
    f32 = np.float32
    features = np.asarray(inputs["features"], f32)
    idx = np.asarray(inputs["idx"]).astype(np.int32)
    n_total = features.shape[0]
    R = n_total // n_cores
    B = R // 128

    smalls = np.concatenate(
        [
            np.asarray(inputs["jnt_err"], f32),
            np.asarray(inputs["jnt_dedt"], f32),
            np.asarray(inputs["weights"], f32),
            np.ones((n_total, 1), f32),
            np.asarray(inputs["actions"], f32),
        ],
        axis=1,
    )  # [N, 22]

    W1a = np.asarray(inputs["W1a"], f32)
    W1c = np.asarray(inputs["W1c"], f32)
    w1a_big = np.ascontiguousarray(W1a[15:527])
    w1c_big = np.ascontiguousarray(W1c[21:533])
    w1a_small = np.concatenate([W1a[0:15], np.asarray(inputs["b1a"], f32)[None, :]], 0)
    w1c_small = np.concatenate(
        [W1c[0:15], np.asarray(inputs["b1c"], f32)[None, :], W1c[15:21]], 0
    )

    shared = {
        "w1a_big": w1a_big,
        "w1c_big": w1c_big,
        "w1a_small": np.ascontiguousarray(w1a_small),
        "w1c_small": np.ascontiguousarray(w1c_small),
        "w2a": np.asarray(inputs["W2a"], f32),
        "w2c": np.asarray(inputs["W2c"], f32),
        "b2a": np.asarray(inputs["b2a"], f32),
        "b2c": np.asarray(inputs["b2c"], f32),
        "g1a": np.asarray(inputs["g1a"], f32),
        "be1a": np.asarray(inputs["be1a"], f32),
        "g1c": np.asarray(inputs["g1c"], f32),
        "be1c": np.asarray(inputs["be1c"], f32),
    }

    featT = np.ascontiguousarray(features.T)  # [512, N]
    smallsT = np.ascontiguousarray(smalls.T)  # [22, N]

    in_maps = []
    for c in range(n_cores):
        sl = slice(c * R, (c + 1) * R)
        tok = idx[sl]
        m = dict(shared)
        m["featT"] = np.ascontiguousarray(featT[:, sl])
        m["smallsT"] = np.ascontiguousarray(smallsT[:, sl])
        m["idx_tok"] = np.ascontiguousarray(tok.reshape(B, 128).T)
        in_maps.append(m)
    return in_maps


def _is_identity_ln(inputs):
    return (
        np.all(np.asarray(inputs["g1a"]) == 1.0)
        and np.all(np.asarray(inputs["be1a"]) == 0.0)
        and np.all(np.asarray(inputs["g1c"]) == 1.0)
        and np.all(np.asarray(inputs["be1c"]) == 0.0)
    )


def kernel(**inputs):
    n_cores = 8
    n_total = np.asarray(inputs["features"]).shape[0]
    general_ln = not _is_identity_ln(inputs)

    key = (n_total, n_cores, general_ln)
    if key not in _BUILD_CACHE:
        _BUILD_CACHE[key] = build_nc(n_total, n_cores, general_ln=general_ln)
    nc = _BUILD_CACHE[key]

    in_maps = _prep_host(inputs, n_cores)
    res = run_bass_kernel_spmd(nc, in_maps, core_ids=list(range(n_cores)))
    out = np.concatenate([r["out7"] for r in res.results], axis=1)  # [7, N]
    act_out = np.ascontiguousarray(out[:6].T)
    crit_out = np.ascontiguousarray(out[6:7].T)
    return act_out, crit_out


if __name__ == "__main__":
    nc = build_nc(131072, 8)
    print("build ok:", len(nc.inst_map), "instructions")
    from concourse.timeline_sim import TimelineSim
    print("TimelineSim ns:", TimelineSim(nc).simulate())


# revision 47
# speedup vs baseline: 1.0008x; 1.0008x over previous
# Bass/Trainium2 kernel for nn_ActorCritic (scatter_memory):
#   x = zeros_like(features).at[idx].set(features[idx])  == features * mask,
#       mask[r] = (r in idx)
#   act_out  = tanh(selu(LN(cat(err,dedt,w,x) @ W1a + b1a)) @ W2a + b2a)
#   crit_out =      selu(LN(cat(err,dedt,w,act,x) @ W1c + b1c)) @ W2c + b2c
#
# 8-core data-parallel: rows sharded, weights replicated.  The membership
# mask is built on-device with a race-free TensorEngine histogram: per
# 128-token chunk of the core's idx shard, one-hot(idx&127) is the
# stationary operand and one-hot(idx>>7) the moving operand, accumulated in
# PSUM over the whole shard; a ReduceScatter(add) sums the 8 partial count
# tables and hands each core its own row range, and mask = min(count, 1).
#
# The MLP: layer-1 matmuls in float32r (full PE rate at 512-wide moving
# operand), LN bias b1 folded in via an appended ones-column, SELU via
# exp/relu/min with the lambda folded into W2, layer 2 in fp16 with the
# activations transposed by the DMA xbar.  LN sqrt is batched per tile
# group so the ACT engine loads the sqrt table twice per group instead of
# per tile.
import numpy as np

import concourse.mybir as mybir
import concourse.tile as tile
from concourse import bacc
from concourse.bass_utils import run_bass_kernel_spmd
from concourse.masks import make_identity

F32 = mybir.dt.float32
F32R = mybir.dt.float32r
F16 = mybir.dt.float16
F8 = mybir.dt.float8e4
BF16 = mybir.dt.bfloat16
I32 = mybir.dt.int32
I16 = mybir.dt.int16

SELU_ALPHA = 1.6732632423543772848170429916717
SELU_LAMBDA = 1.0507009873554804934193349852946

FEAT = 512
LIN = 512
NSMALL = 22  # err(6) dedt(6) w(3) ones(1) actions(6)
K_ACT = 16   # actor uses rows 0:16 of smallsT (incl. ones)
K_CRIT = 22  # critic uses all 22

_BUILD_CACHE = {}


def build_nc(n_total, n_cores, group_tiles=8, general_ln=False):
    """Build the SPMD Bacc graph for one core (identical across cores)."""
    R = n_total // n_cores          # rows per core
    assert R % 128 == 0
    T = R // 128                    # 128-row tiles per core
    B = R // 128                    # idx-shard chunks (tokens laid [128, B])
    G = min(group_tiles, T)
    assert T % G == 0
    NG = T // G
    GC = G * 128                    # columns per group

    nc = bacc.Bacc()

    # ---------------- DRAM parameters ----------------
    featT = nc.dram_tensor("featT", [FEAT, R], F32, kind="ExternalInput")
    smallsT = nc.dram_tensor("smallsT", [NSMALL, R], F32, kind="ExternalInput")
    idx_tok = nc.dram_tensor("idx_tok", [128, B], I32, kind="ExternalInput")
    w1a_big = nc.dram_tensor("w1a_big", [FEAT, LIN], F32, kind="ExternalInput")
    w1c_big = nc.dram_tensor("w1c_big", [FEAT, LIN], F32, kind="ExternalInput")
    w1a_small = nc.dram_tensor("w1a_small", [K_ACT, LIN], F32, kind="ExternalInput")
    w1c_small = nc.dram_tensor("w1c_small", [K_CRIT, LIN], F32, kind="ExternalInput")
    w2a_in = nc.dram_tensor("w2a", [LIN, 6], F32, kind="ExternalInput")
    w2c_in = nc.dram_tensor("w2c", [LIN, 1], F32, kind="ExternalInput")
    b2a_in = nc.dram_tensor("b2a", [6], F32, kind="ExternalInput")
    b2c_in = nc.dram_tensor("b2c", [1], F32, kind="ExternalInput")
    g1a_in = nc.dram_tensor("g1a", [LIN], F32, kind="ExternalInput")
    be1a_in = nc.dram_tensor("be1a", [LIN], F32, kind="ExternalInput")
    g1c_in = nc.dram_tensor("g1c", [LIN], F32, kind="ExternalInput")
    be1c_in = nc.dram_tensor("be1c", [LIN], F32, kind="ExternalInput")

    out7 = nc.dram_tensor("out7", [7, R], F32, kind="ExternalOutput")

    # histogram table (Local) and reduce-scatter output (this core's rows)
    cnt_tbl = nc.dram_tensor("cnt_tbl", [n_total], F32)
    cnt_rs = nc.dram_tensor("cnt_rs", [R], F32)

    with tile.TileContext(nc) as tc:
        with (
            tc.tile_pool(name="consts", bufs=1) as consts,
        ):
            # ---- weights / constants ----
            wa = consts.tile([128, 4, LIN], F32R, tag="wa")
            nc.gpsimd.dma_start(out=wa[:], in_=w1a_big.ap().rearrange("(f p) n -> p f n", p=128))
            wc = consts.tile([128, 4, LIN], F32R, tag="wc")
            nc.gpsimd.dma_start(out=wc[:], in_=w1c_big.ap().rearrange("(f p) n -> p f n", p=128))
            wsa = consts.tile([K_ACT, LIN], F32R, tag="wsa")
            nc.gpsimd.dma_start(out=wsa[:], in_=w1a_small[:, :])
            wsc = consts.tile([K_CRIT, LIN], F32R, tag="wsc")
            nc.gpsimd.dma_start(out=wsc[:], in_=w1c_small[:, :])
            w2a = consts.tile([128, 4, 6], F16, tag="w2a")
            nc.gpsimd.dma_start(out=w2a[:], in_=w2a_in.ap().rearrange("(f p) j -> p f j", p=128))
            nc.vector.tensor_scalar_mul(w2a[:], w2a[:], SELU_LAMBDA)
            w2c = consts.tile([128, 4, 1], F16, tag="w2c")
            nc.gpsimd.dma_start(out=w2c[:], in_=w2c_in.ap().rearrange("(f p) j -> p f j", p=128))
            nc.vector.tensor_scalar_mul(w2c[:], w2c[:], SELU_LAMBDA)
            b2a = consts.tile([6, 1], F32, tag="b2a")
            nc.gpsimd.dma_start(out=b2a[:], in_=b2a_in[:, None])
            b2c = consts.tile([1, 1], F32, tag="b2c")
            nc.gpsimd.dma_start(out=b2c[:], in_=b2c_in[:, None])
            epsb = consts.tile([128, 1], F32, tag="epsb")
            nc.vector.memset(epsb[:], 1e-5)

            if general_ln:
                g_a = consts.tile([128, LIN], F16, tag="g_a")
                nc.gpsimd.dma_start(out=g_a[:], in_=g1a_in[None, :].to_broadcast([128, LIN]))
                be_a = consts.tile([128, LIN], F16, tag="be_a")
                nc.gpsimd.dma_start(out=be_a[:], in_=be1a_in[None, :].to_broadcast([128, LIN]))
                g_c = consts.tile([128, LIN], F16, tag="g_c")
                nc.gpsimd.dma_start(out=g_c[:], in_=g1c_in[None, :].to_broadcast([128, LIN]))
                be_c = consts.tile([128, LIN], F16, tag="be_c")
                nc.gpsimd.dma_start(out=be_c[:], in_=be1c_in[None, :].to_broadcast([128, LIN]))

            # ---------------- phase A: membership mask ----------------
            # Race-free histogram of the idx shard over all N rows:
            # counts[lo, hi] accumulated on the TensorEngine in PSUM.
            HI = n_total // 128
            with (
                tc.tile_pool(name="phA", bufs=1) as pa,
                tc.tile_pool(name="phAw", bufs=4) as paw,
                tc.tile_pool(name="psCnt", bufs=1, space="PSUM") as psCnt,
            ):
                v = pa.tile([128, B], I32, tag="v")
                nc.gpsimd.dma_start(out=v[:], in_=idx_tok[:, :])
                vhi_i = pa.tile([128, B], I32, tag="vhi_i")
                nc.vector.tensor_scalar(
                    vhi_i[:], v[:], 7, None, mybir.AluOpType.logical_shift_right)
                vhi = pa.tile([128, B], F32, tag="vhi")
                nc.vector.tensor_copy(vhi[:], vhi_i[:])
                vlo_i = pa.tile([128, B], I32, tag="vlo_i")
                nc.vector.tensor_scalar(
                    vlo_i[:], v[:], 127, None, mybir.AluOpType.bitwise_and)
                vlo = pa.tile([128, B], F32, tag="vlo")
                nc.vector.tensor_copy(vlo[:], vlo_i[:])
                iota_hi_i = pa.tile([128, HI], I16, tag="iota_hi_i")
                nc.gpsimd.iota(iota_hi_i[:], pattern=[[1, HI]], base=0,
                               channel_multiplier=0)
                iota_hi = pa.tile([128, HI], F16, tag="iota_hi")
                nc.vector.tensor_copy(iota_hi[:], iota_hi_i[:])
                iota_lo_i = pa.tile([128, 128], I16, tag="iota_lo_i")
                nc.gpsimd.iota(iota_lo_i[:], pattern=[[1, 128]], base=0,
                               channel_multiplier=0)
                iota_lo = pa.tile([128, 128], F16, tag="iota_lo")
                nc.vector.tensor_copy(iota_lo[:], iota_lo_i[:])

                cnt_ps = psCnt.tile([128, HI], F32, tag="cnt_ps")
                for j in range(B):
                    a_t = paw.tile([128, HI], F16, tag="a_t")
                    nc.vector.tensor_scalar(
                        a_t[:], iota_hi[:], vhi[:, j:j + 1], None,
                        mybir.AluOpType.is_equal)
                    b_t = paw.tile([128, 128], F16, tag="b_t")
                    nc.vector.tensor_scalar(
                        b_t[:], iota_lo[:], vlo[:, j:j + 1], None,
                        mybir.AluOpType.is_equal)
                    MMW = min(512, HI)
                    for h in range(HI // MMW):
                        nc.tensor.matmul(
                            cnt_ps[:, h * MMW:(h + 1) * MMW],
                            lhsT=b_t[:], rhs=a_t[:, h * MMW:(h + 1) * MMW],
                            start=(j == 0), stop=(j == B - 1),
                        )
                # counts to DRAM in row-major order: transpose each 128-wide
                # hi-block so DMA runs are 512B-contiguous
                cnt_sb = pa.tile([128, HI], F32, tag="cnt_sb")
                nc.scalar.copy(cnt_sb[:], cnt_ps[:])
                identf = pa.tile([128, 128], F32, tag="identf")
                make_identity(nc, identf[:])
                HW_ = min(128, HI)
                n_h = HI // HW_
                cntT = pa.tile([HW_, n_h, 128], F32, tag="cntT")
                for h in range(n_h):
                    ctp = psCnt.tile([HW_, 128], F32, tag="ctp")
                    nc.tensor.transpose(
                        ctp[:], cnt_sb[:, h * HW_:(h + 1) * HW_], identf[:])
                    nc.vector.tensor_copy(cntT[:, h, :], ctp[:])
                nc.gpsimd.dma_start(
                    out=cnt_tbl.ap().rearrange("(h hi lo) -> hi h lo", hi=HW_, lo=128),
                    in_=cntT[:],
                )
                # sum partial counts across cores; each core keeps its row range
                nc.gpsimd.collective_compute(
                    "ReduceScatter",
                    mybir.AluOpType.add,
                    replica_groups=[list(range(n_cores))],
                    ins=[cnt_tbl.ap()],
                    outs=[cnt_rs.ap()],
                )

            # ---------------- main loop ----------------
            with (
                tc.tile_pool(name="xg", bufs=3) as xgp,
                tc.tile_pool(name="sg", bufs=3) as sgp,
                tc.tile_pool(name="og", bufs=2) as ogp,
                tc.tile_pool(name="hsb", bufs=2 * G + 2) as hsbp,
                tc.tile_pool(name="wk", bufs=2) as wk,
                tc.tile_pool(name="wks", bufs=G + 2) as wks,
                tc.tile_pool(name="psH", bufs=2, space="PSUM") as psH,
                tc.tile_pool(name="psO", bufs=2, space="PSUM") as psO,
            ):
                for g in range(NG):
                    gsl = slice(g * GC, (g + 1) * GC)
                    xg = xgp.tile([128, 4, GC], F32R, tag="xg")
                    nc.gpsimd.dma_start(
                        out=xg[:],
                        in_=featT.ap().rearrange("(f p) r -> p f r", p=128)[:, :, gsl],
                    )
                    sg = sgp.tile([NSMALL, GC], F32R, tag="sg")
                    nc.gpsimd.dma_start(out=sg[:], in_=smallsT[:, gsl])
                    oga = ogp.tile([6, GC], F32, tag="oga")
                    ogc = ogp.tile([1, GC], F32, tag="ogc")
                    # mask the x part (rows with no idx hit contribute zero):
                    # broadcast-load this group's counts, mask = min(count, 1)
                    cr = sgp.tile([128, GC], F32, tag="cr")
                    nc.sync.dma_start(
                        out=cr[:], in_=cnt_rs[None, gsl].to_broadcast([128, GC]))
                    mg = sgp.tile([128, GC], F32R, tag="mg")
                    nc.vector.tensor_scalar(
                        mg[:], cr[:], 1.0, None, mybir.AluOpType.min)
                    for f in range(4):
                        nc.vector.tensor_tensor(
                            out=xg[:, f, :], in0=xg[:, f, :], in1=mg[:],
                            op=mybir.AluOpType.mult,
                        )

                    # -- sweep 1: layer 1 + psum eviction + LN stats --
                    agg_g = wks.tile([128, 4 * G], F32, tag="agg_g")
                    hs = {}
                    for t in range(G):
                        tsl = slice(t * 128, (t + 1) * 128)
                        psA = psH.tile([128, LIN], F32, tag="psA")
                        psC = psH.tile([128, LIN], F32, tag="psC")
                        for f in range(4):
                            nc.tensor.matmul(
                                psA[:], lhsT=xg[:, f, tsl], rhs=wa[:, f, :],
                                start=(f == 0), stop=False,
                            )
                        nc.tensor.matmul(
                            psA[:], lhsT=sg[:K_ACT, tsl], rhs=wsa[:],
                            start=False, stop=True,
                        )
                        for f in range(4):
                            nc.tensor.matmul(
                                psC[:], lhsT=xg[:, f, tsl], rhs=wc[:, f, :],
                                start=(f == 0), stop=False,
                            )
                        nc.tensor.matmul(
                            psC[:], lhsT=sg[:K_CRIT, tsl], rhs=wsc[:],
                            start=False, stop=True,
                        )
                        for net, ps in (("a", psA), ("c", psC)):
                            h = hsbp.tile([128, LIN], F16, tag=f"h{net}")
                            nc.scalar.copy(h[:], ps[:])
                            st6 = wk.tile([128, 6], F32, tag=f"st6{net}")
                            nc.vector.bn_stats(st6[:], h[:])
                            off = 4 * t + (0 if net == "a" else 2)
                            nc.vector.bn_aggr(agg_g[:, off:off + 2], st6[:])
                            hs[(t, net)] = h

                    # -- sweep 2: group-wide LN scalars --
                    # sd over the strided var columns (one Sqrt instruction,
                    # so the ACT table switches only twice per group)
                    sd_g = wks.tile([128, 2 * G], F32, tag="sd_g")
                    nc.scalar.activation(
                        sd_g[:], agg_g[:].rearrange("p (t two) -> p (t two)", two=2)[
                            :, 1::2],
                        mybir.ActivationFunctionType.Sqrt,
                        bias=epsb[:], scale=1.0,
                    )
                    k_g = wks.tile([128, 2 * G], F32, tag="k_g")
                    nc.vector.reciprocal(k_g[:], sd_g[:])
                    # nmk = -mean * k for all tiles/nets at once
                    nmk_g = wks.tile([128, 2 * G], F32, tag="nmk_g")
                    nc.vector.tensor_tensor(
                        nmk_g[:], agg_g[:].rearrange("p (t two) -> p (t two)", two=2)[
                            :, 0::2],
                        k_g[:], op=mybir.AluOpType.mult)
                    nc.vector.tensor_scalar(
                        nmk_g[:], nmk_g[:], -1.0, None, mybir.AluOpType.mult)

                    # -- sweep 3: selu(LN(h)) and layer 2, two tiles per
                    # layer-2 matmul batch (N=256 moving operand) --
                    for tp in range(G // 2):
                        psl = slice(tp * 256, (tp + 1) * 256)
                        out2a = psO.tile([6, 256], F32, tag="out2a")
                        out2c = psO.tile([1, 256], F32, tag="out2c")
                        haTpa = wk.tile([128, 4, 256], F16, tag="haTpa")
                        haTpc = wk.tile([128, 4, 256], F16, tag="haTpc")
                        haTp = {"a": haTpa, "c": haTpc}
                        for ti in range(2):
                          t = 2 * tp + ti
                          for net in ("a", "c"):
                            h = hs[(t, net)]
                            col = 2 * t + (0 if net == "a" else 1)
                            k = k_g[:, col:col + 1]
                            nmk = nmk_g[:, col:col + 1]
                            ti_sl = slice(ti * 128, (ti + 1) * 128)
                            if general_ln:
                                z_t = wk.tile([128, LIN], F16, tag=f"z{net}")
                                nc.scalar.activation(
                                    z_t[:], h[:], mybir.ActivationFunctionType.Identity,
                                    bias=nmk, scale=k,
                                )
                                gg = g_a if net == "a" else g_c
                                bb = be_a if net == "a" else be_c
                                nc.vector.tensor_tensor(
                                    z_t[:], z_t[:], gg[:], op=mybir.AluOpType.mult)
                                nc.vector.tensor_tensor(
                                    z_t[:], z_t[:], bb[:], op=mybir.AluOpType.add)
                                e_t = wk.tile([128, LIN], F16, tag=f"e{net}")
                                nc.scalar.activation(
                                    e_t[:], z_t[:], mybir.ActivationFunctionType.Exp,
                                    bias=0.0, scale=1.0)
                                r_t = wk.tile([128, LIN], F16, tag=f"r{net}")
                                nc.vector.tensor_scalar(
                                    r_t[:], z_t[:], 0.0, -SELU_ALPHA,
                                    mybir.AluOpType.max, mybir.AluOpType.add)
                            else:
                                e_t = wk.tile([128, LIN], F16, tag=f"e{net}")
                                nc.scalar.activation(
                                    e_t[:], h[:], mybir.ActivationFunctionType.Exp,
                                    bias=nmk, scale=k,
                                )
                                # r = max(k*h + nmk, 0) - alpha on the DVE
                                r_t = wk.tile([128, LIN], F16, tag=f"r{net}")
                                nc.vector.tensor_scalar(
                                    r_t[:], h[:], k, nmk,
                                    mybir.AluOpType.mult, mybir.AluOpType.add)
                                nc.vector.tensor_scalar(
                                    r_t[:], r_t[:], 0.0, -SELU_ALPHA,
                                    mybir.AluOpType.max, mybir.AluOpType.add)
                            # ha/lambda = (relu - alpha) + alpha*min(e, 1)
                            nc.vector.tensor_scalar(
                                e_t[:], e_t[:], 1.0, SELU_ALPHA,
                                mybir.AluOpType.min, mybir.AluOpType.mult)
                            ha = wk.tile([128, LIN], F16, tag=f"ha{net}")
                            nc.vector.tensor_tensor(
                                ha[:], r_t[:], e_t[:], op=mybir.AluOpType.add)
                            # transpose via the DMA xbar into this tile's half
                            # of the pair buffer: haT[p, c, r] = ha[r, 128c+p]
                            nc.sync.dma_start_transpose(
                                out=haTp[net][:, :, ti_sl], in_=ha[:])
                        for net in ("a", "c"):
                            w2 = w2a if net == "a" else w2c
                            o2 = out2a if net == "a" else out2c
                            for f in range(4):
                                nc.tensor.matmul(
                                    o2[:], lhsT=w2[:, f, :], rhs=haTp[net][:, f, :],
                                    start=(f == 0), stop=(f == 3),
                                )
                        nc.scalar.activation(
                            oga[:, psl], out2a[:],
                            mybir.ActivationFunctionType.Tanh, bias=b2a[:], scale=1.0,
                        )
                        nc.scalar.activation(
                            ogc[:, psl], out2c[:],
                            mybir.ActivationFunctionType.Identity, bias=b2c[:], scale=1.0,
                        )
                    nc.sync.dma_start(out=out7[0:6, gsl], in_=oga[:])
                    nc.sync.dma_start(out=out7[6:7, gsl], in_=ogc[:])

    nc.compile()
    return nc


def _prep_host(inputs, n_cores):
    """Layout-only host prep: shard + transpose + permute, no arithmetic on
    data values (the ones column and dtype casts are the only additions)."""
    f32 = np.float32
    features = np.asarray(inputs["features"], f32)
    idx = np.asarray(inputs["idx"]).astype(np.int32)
    n_total = features.shape[0]
    R = n_total // n_cores
    B = R // 128

    smalls = np.concatenate(
        [
            np.asarray(inputs["jnt_err"], f32),
            np.asarray(inputs["jnt_dedt"], f32),
            np.asarray(inputs["weights"], f32),
            np.ones((n_total, 1), f32),
            np.asarray(inputs["actions"], f32),
        ],
        axis=1,
    )  # [N, 22]

    W1a = np.asarray(inputs["W1a"], f32)
    W1c = np.asarray(inputs["W1c"], f32)
    w1a_big = np.ascontiguousarray(W1a[15:527])
    w1c_big = np.ascontiguousarray(W1c[21:533])
    w1a_small = np.concatenate([W1a[0:15], np.asarray(inputs["b1a"], f32)[None, :]], 0)
    w1c_small = np.concatenate(
        [W1c[0:15], np.asarray(inputs["b1c"], f32)[None, :], W1c[15:21]], 0
    )

    shared = {
        "w1a_big": w1a_big,
        "w1c_big": w1c_big,
        "w1a_small": np.ascontiguousarray(w1a_small),
        "w1c_small": np.ascontiguousarray(w1c_small),
        "w2a": np.asarray(inputs["W2a"], f32),
        "w2c": np.asarray(inputs["W2c"], f32),
        "b2a": np.asarray(inputs["b2a"], f32),
        "b2c": np.asarray(inputs["b2c"], f32),
        "g1a": np.asarray(inputs["g1a"], f32),
        "be1a": np.asarray(inputs["be1a"], f32),
        "g1c": np.asarray(inputs["g1c"], f32),
        "be1c": np.asarray(inputs["be1c"], f32),
    }

    featT = np.ascontiguousarray(features.T)  # [512, N]
    smallsT = np.ascontiguousarray(smalls.T)  # [22, N]

    in_maps = []
    for c in range(n_cores):
        sl = slice(c * R, (c + 1) * R)
        tok = idx[sl]
        m = dict(shared)
        m["featT"] = np.ascontiguousarray(featT[:, sl])
        m["smallsT"] = np.ascontiguousarray(smallsT[:, sl])
        m["idx_tok"] = np.ascontiguousarray(tok.reshape(B, 128).T)
        in_maps.append(m)
    return in_maps


def _is_identity_ln(inputs):
    return (
        np.all(np.asarray(inputs["g1a"]) == 1.0)
        and np.all(np.asarray(inputs["be1a"]) == 0.0)
        and np.all(np.asarray(inputs["g1c"]) == 1.0)
        and np.all(np.asarray(inputs["be1c"]) == 0.0)
    )


def kernel(**inputs):
    n_cores = 8
    n_total = np.asarray(inputs["features"]).shape[0]
    general_ln = not _is_identity_ln(inputs)

    key = (n_total, n_cores, general_ln)
    if key not in _BUILD_CACHE:
        _BUILD_CACHE[key] = build_nc(n_total, n_cores, general_ln=general_ln)
    nc = _BUILD_CACHE[key]

    in_maps = _prep_host(inputs, n_cores)
    res = run_bass_kernel_spmd(nc, in_maps, core_ids=list(range(n_cores)))
    out = np.concatenate([r["out7"] for r in res.results], axis=1)  # [7, N]
    act_out = np.ascontiguousarray(out[:6].T)
    crit_out = np.ascontiguousarray(out[6:7].T)
    return act_out, crit_out


if __name__ == "__main__":
    nc = build_nc(131072, 8)
    print("build ok:", len(nc.inst_map), "instructions")
    from concourse.timeline_sim import TimelineSim
    print("TimelineSim ns:", TimelineSim(nc).simulate())
